# revision 7
# baseline (speedup 1.0000x reference)
"""BitLinear (fake-quant straight-through) Trainium2 kernel.

Math (per the reference nn module):
  dqx = round(x * s_x) / s_x         s_x = 127 / clip(rowabsmax(x), 1e-5)   (per token row)
  dqw = clip(round(w * s_w), -1, 1) / s_w    s_w = 1 / clip(mean(|w|), 1e-5)  (per tensor)
  out = dqx @ dqw.T + bias

Key facts this kernel exploits:
  * round(x*s_x) is an integer in [-127, 127] and clip(round(w*s_w)) is in
    {-1, 0, 1}; both are EXACT in bf16, and the matmul accumulates in fp32
    PSUM where all partial sums (<= 2^17) are exact integers.  So the heavy
    matmul runs at bf16 PE rate with zero quantization-path error; the
    per-token / per-tensor scales are applied to the (exact) integer matmul
    result at PSUM evacuation.
  * round-half-even == fp32 RNE, so `round(v)` is computed exactly as
    `(v + 1.5*2^23) - 1.5*2^23` with two fp32 ALU stages (no Round op needed).
  * The weight-side transform (ternary quantize + [k-on-partitions]
    transpose + bf16 cast) depends only on `weight`: it is done once on the
    host with bit-exact fp32 numpy ops (np.rint == RNE == jnp.round) and
    shipped as a 2 MiB bf16 input, removing the whole on-device weight-prep
    stage and 2 MiB of HBM traffic.

Sharding: data parallel over the batch dim; core i computes batch element i
with the full weight.  No collectives; the host scatters x and gathers out.

Pipeline structure: tokens are processed in "quads" (4 x 128 = 512 tokens).
Per quad: one 2 MiB x load, one absmax reduce, round via magic constant,
one batched xbar transpose ([128, 4096]bf16 -> [128, 4, 8, 128]), and 64
back-to-back 512-wide matmuls.  Evacuation fuses the per-token output scale
and the bias add in a single scalar_tensor_tensor op reading PSUM.

Engine assignment (each stage owns an engine so stages only queue behind
themselves; the three DMA streams use three different DMA rings):
  scalar (ACT HWDGE ring) : x input loads
  sync   (SP HWDGE ring)  : xbar transposes
  gpsimd (SWDGE ring)     : round (x*ss + MAGIC), -MAGIC + bf16 cast,
                            output stores
  vector : absmax reduce, scales, fused PSUM evac (scale*psum + bias)
  tensor : matmuls (bf16 exact-integer)

The per-tensor weight scale s_w is computed on the host (it must match the
reference's fp32 mean reduction to ~1 ulp); the derived output scale factor
k1 = (1/s_w)/127 is passed through a small constants tensor, so the compiled
program is input-independent.
"""

import numpy as np

from concourse import bacc, bass, mybir, tile
from concourse.bass_utils import run_bass_kernel_spmd

F32 = mybir.dt.float32
BF16 = mybir.dt.bfloat16
ALU = mybir.AluOpType
ACTF = mybir.ActivationFunctionType

MAGIC = 12582912.0  # 1.5 * 2**23: fp32 RNE round-to-integer constant
EPS = 1e-05

B, S, K, N = 8, 4096, 1024, 1024
N_CORES = 8
QS = 4  # token tiles per quad


def build(s_tokens=S, k=K, n=N):
    """Build the single-core SPMD program: x[s_tokens,k] @ w[n,k]^T quantized."""
    nc = bacc.Bacc("TRN2", target_bir_lowering=False, debug=False)

    KT = k // 128          # contraction tiles
    NT = n // 128          # weight row tiles
    NH = n // 512          # psum-bank halves of the output feature dim
    NQ = s_tokens // (128 * QS)  # quads

    x_d = nc.dram_tensor("x", [s_tokens, k], F32, kind="ExternalInput").ap()
    # pre-quantized, pre-transposed ternary weight (host):
    # qwt[p, nt, kt, j] = ternary(w)[nt*128+j, kt*128+p]
    qwt_d = nc.dram_tensor("qwt", [128, NT, KT, 128], BF16, kind="ExternalInput").ap()
    # bias broadcast to all 128 partitions (host)
    bias_d = nc.dram_tensor("biasb", [128, n], F32, kind="ExternalInput").ap()
    consts_d = nc.dram_tensor("consts", [128, 2], F32, kind="ExternalInput").ap()
    out_d = nc.dram_tensor("out", [s_tokens, n], F32, kind="ExternalOutput").ap()

    x_q = x_d.rearrange("(q s p) k -> q p s k", s=QS, p=128)
    out_q = out_d.rearrange("(q s p) n -> q p s n", s=QS, p=128)

    with tile.TileContext(nc) as tc:
        with (
            tc.tile_pool(name="static", bufs=1) as static,
            tc.tile_pool(name="xpool", bufs=4) as xpool,
            tc.tile_pool(name="qpool", bufs=3) as qpool,
            tc.tile_pool(name="qtpool", bufs=3) as qtpool,
            tc.tile_pool(name="opool", bufs=2) as opool,
            tc.tile_pool(name="vpool", bufs=6) as vpool,
            tc.tile_pool(name="psum", bufs=3, space="PSUM") as psum_pool,
        ):
            # static loads ride the (otherwise idle-at-start) SWDGE ring so
            # the scalar ring's first x load starts at t=0
            consts = static.tile([128, 2], F32)
            nc.gpsimd.dma_start(consts[:], consts_d[:])
            bias_sb = static.tile([128, n], F32)
            nc.gpsimd.dma_start(bias_sb[:], bias_d[:])
            # qwT[kpart, nt, kt, n128]: quantized weight, k on partitions
            qwT = static.tile([128, NT, KT, 128], BF16)
            nc.gpsimd.dma_start(qwT[:], qwt_d[:])

            k1 = consts[:, 0:1]       # (1/s_w) / 127  (output scale factor)

            # ---- software-pipelined emission over token quads ----
            # preload(q) = x DMA (3 quads ahead, no data deps);
            # pre(q) = quantize + transpose, in 2 half-quad slices for
            # latency; mm(q) = matmuls + fused evac + store.
            # Emission order per iteration: mm(q), preload(q+3), pre(q+2) —
            # mm(q) first so the PE drain path (evac on DVE) is never queued
            # behind load-dependent work; pre 2 ahead gives every cross-
            # engine dependency ~2 quad-periods of slack instead of
            # collapsing the pipeline to depth ~1.
            HS = QS // 2  # tiles per half-quad slice
            xss, qxTs, fss = {}, {}, {}

            def preload(q):
                x_s = xpool.tile([128, QS, k], F32, name="x_s")
                for u in range(2):
                    nc.scalar.dma_start(x_s[:, u * HS:(u + 1) * HS, :],
                                        x_q[q][:, u * HS:(u + 1) * HS, :])
                xss[q] = x_s

            def pre(q):
                x_s = xss.pop(q)
                qxT = qtpool.tile([128, QS, KT, 128], BF16, name="qxT")
                fs_q = []
                for u in range(2):
                    xu = x_s[:, u * HS:(u + 1) * HS, :]
                    c = vpool.tile([128, HS], F32, name="c")
                    nc.vector.tensor_reduce(
                        c[:], xu, mybir.AxisListType.X, ALU.max,
                        apply_absolute_value=True,
                    )
                    cc = vpool.tile([128, HS], F32, name="cc")
                    nc.vector.tensor_scalar_max(cc[:], c[:], EPS)
                    rc = vpool.tile([128, HS], F32, name="rc")
                    nc.vector.reciprocal(rc[:], cc[:])
                    ss = vpool.tile([128, HS], F32, name="ss")
                    nc.vector.tensor_scalar_mul(ss[:], rc[:], 127.0)
                    fs = vpool.tile([128, HS], F32, name="fs")
                    nc.vector.tensor_scalar_mul(fs[:], cc[:], k1)
                    fs_q.append(fs)

                    # round(x*s_x) via magic constant, in place (gpsimd)
                    for j in range(HS):
                        nc.gpsimd.tensor_scalar(
                            x_s[:, u * HS + j, :], x_s[:, u * HS + j, :],
                            ss[:, j:j + 1], MAGIC, ALU.mult, ALU.add,
                        )
                    # -MAGIC + bf16 cast on ACT (y - MAGIC is Sterbenz-exact,
                    # so ACT's fused affine adds no extra rounding; gpsimd's
                    # f32->bf16 cast ucode path is ~50x too slow to use here)
                    qx = qpool.tile([128, HS, k], BF16, name="qx")
                    nc.scalar.activation(qx[:], xu, ACTF.Copy, bias=-MAGIC)

                    # xbar transpose of the half-quad:
                    # [128s, HS*k]bf16 -> [128k, HS, KT, 128s], chunk j*KT+kt
                    nc.sync.dma_start_transpose(
                        qxT[:, u * HS:(u + 1) * HS], qx[:]
                    )
                qxTs[q] = qxT
                fss[q] = fs_q

            def mm_and_store(q):
                qxT, fs_q = qxTs.pop(q), fss.pop(q)
                outs = opool.tile([128, QS, n], F32, name="outs")
                for s in range(QS):
                    fs = fs_q[s // HS]
                    fcol = s % HS
                    ps_list = [
                        psum_pool.tile([128, 512], F32, name=f"ps{h}", tag=f"ps{h}")
                        for h in range(NH)
                    ]
                    for kt in range(KT):
                        for h in range(NH):
                            nc.tensor.matmul(
                                ps_list[h][:],
                                qxT[:, s, kt, :],
                                qwT[:, 4 * h:4 * h + 4, kt, :],
                                start=(kt == 0),
                                stop=(kt == KT - 1),
                            )
                    # fused evac: outs = psum * fs[s] + bias
                    for h in range(NH):
                        nc.vector.scalar_tensor_tensor(
                            outs[:, s, h * 512:(h + 1) * 512],
                            ps_list[h][:],
                            fs[:, fcol:fcol + 1],
                            bias_sb[:, h * 512:(h + 1) * 512],
                            ALU.mult,
                            ALU.add,
                        )
                nc.gpsimd.dma_start(out_q[q], outs[:])

            for q in range(min(3, NQ)):
                preload(q)
            pre(0)
            if NQ > 1:
                pre(1)
            for q in range(NQ):
                mm_and_store(q)
                if q + 3 < NQ:
                    preload(q + 3)
                if q + 2 < NQ:
                    pre(q + 2)

    nc.compile()
    return nc


def host_weight(weight):
    """Bit-exact host-side ternary quantization + transpose of the weight.

    Matches the reference: scale = 1/clip(mean|w|, eps) in jax fp32;
    clip(round(w*scale), -1, 1).  np.rint is RNE == jnp.round.
    Returns qwt[p, nt, kt, j] = tern[nt*128+j, kt*128+p] in bf16.
    """
    import ml_dtypes

    w = np.ascontiguousarray(weight, dtype=np.float32)
    try:
        import jax
        import jax.numpy as jnp

        with jax.default_device(jax.devices("cpu")[0]):
            mean_abs = np.float32(
                jax.device_get(jnp.mean(jnp.abs(jnp.asarray(w, dtype=jnp.float32))))
            )
    except Exception:
        mean_abs = np.float32(np.mean(np.abs(w), dtype=np.float32))
    mean_c = np.maximum(mean_abs, np.float32(EPS))
    sw = np.float32(1.0) / mean_c            # s_w, the weight quant scale
    tern = np.clip(np.rint(w * sw), -1.0, 1.0).astype(ml_dtypes.bfloat16)
    NT, KT = N // 128, K // 128
    qwt = np.ascontiguousarray(
        tern.reshape(NT, 128, KT, 128).transpose(3, 0, 2, 1)
    )
    wdiv = np.float32(1.0) / sw              # 1/s_w (the ternary unit value)
    k1 = wdiv / np.float32(127.0)            # output scale = cc * k1
    return qwt, k1


def make_in_maps(x, weight, bias):
    x = np.ascontiguousarray(x, dtype=np.float32)
    bias = np.ascontiguousarray(bias, dtype=np.float32)
    qwt, k1 = host_weight(weight)
    row = np.zeros((2,), np.float32)
    row[0] = k1
    consts = np.tile(row[None, :], (128, 1)).copy()
    biasb = np.tile(bias[None, :], (128, 1)).copy()
    return [
        {"x": x[i], "qwt": qwt, "biasb": biasb, "consts": consts}
        for i in range(N_CORES)
    ]


_NC_CACHE = {}


def _get_nc():
    if "nc" not in _NC_CACHE:
        _NC_CACHE["nc"] = build()
    return _NC_CACHE["nc"]


def kernel(x, weight, bias, **kwargs):
    nc = _get_nc()
    in_maps = make_in_maps(x, weight, bias)
    last_err = None
    for _attempt in range(3):
        try:
            res = run_bass_kernel_spmd(nc, in_maps, list(range(N_CORES)))
            return np.stack([res.results[i]["out"] for i in range(N_CORES)], axis=0)
        except Exception as e:  # transient NRT device errors: retry
            last_err = e
    raise last_err
